# revision 13
# baseline (speedup 1.0000x reference)
# Distributed Trainium2 Bass kernel for nn_AMRPACore (sparse_attention).
#
# Sharding: 8 cores = (batch b in 0..3) x (query-row half h in 0..1).
# Each core handles rows [h*1024, (h+1)*1024) of batch b and produces its
# memory_bias shard plus partial reductions for the scalar outputs.
#
# Math notes (vs the reference):
#  - proj_vals P_w = decayed_w @ V with decayed_w = g^w+1 * hist[W-1-w] + noise_w.
#    Computed as g^(w+1) * [(hist_rev_w + noise_w/g^(w+1)) @ V]; the noise term
#    is an input-independent constant (jax threefry key 42), generated host-side
#    once, pre-transposed and pre-scaled, stored bf16.
#  - M (B,S,S) is never materialized: MV = einsum('bsw,wbst,btd->bsd') reduces to
#    MV = sum_w alpha_w * P_w.
#  - b2 only shifts all alpha logits equally -> cancels in softmax; dropped.
#  - sim_score scaling (1/sqrt(dk)) is folded into the sigmoid scale; the
#    memory_bias 1/sqrt(dk) is folded into K^T.
import sys
for _p in ('/opt/trn_rl_repo', '/root/.axon_site/_ro/trn_rl_repo'):
    if _p not in sys.path:
        sys.path.insert(0, _p)

import math
import numpy as np
import ml_dtypes

from concourse import bacc, bass, tile, mybir
from concourse.bass_utils import run_bass_kernel_spmd

BF16 = ml_dtypes.bfloat16
F32 = mybir.dt.float32
F32R = mybir.dt.float32r
BF = mybir.dt.bfloat16

B, S, DK = 4, 2048, 128
W = 3
D_MLP = 256
GAMMA = 0.9
EPSILON = 0.01
INV_SQRT_DK = 1.0 / math.sqrt(DK)
SH = S // 2          # rows per core
NK = S // 128        # 16 contraction tiles
NST = SH // 128      # 8 s-tiles per core
NC = 8               # cores
AF = mybir.ActivationFunctionType
ALU = mybir.AluOpType


def _build():
    nc = bacc.Bacc("TRN2", target_bir_lowering=False, debug=False, num_devices=NC)
    dp = nc.declare_dram_parameter
    histT = dp("histT", [W, S, SH], F32R, isOutput=False)     # [w, t, s] f32 bits
    noiseT = dp("noiseT", [W, S, SH], BF, isOutput=False)     # [w, t, s] bf16 (pre-scaled)
    v_ext = dp("v", [S, DK], F32R, isOutput=False)
    kt_ext = dp("kt", [DK, S], F32R, isOutput=False)           # K^T
    qt_ext = dp("qt", [DK, SH], F32R, isOutput=False)          # Q shard ^T
    w1t_ext = dp("w1t", [2, DK, D_MLP], F32R, isOutput=False)  # W1^T f-chunks
    b1_ext = dp("b1r", [2, DK, 1], F32, isOutput=False)
    w2t_ext = dp("w2t", [2, DK, 1], F32, isOutput=False)      # W2^T m-chunks
    wpt_ext = dp("wpt", [DK, DK], F32R, isOutput=False)        # w_proj^T
    wmt_ext = dp("wmt", [DK, DK], F32R, isOutput=False)        # w_mem^T
    gb_ext = dp("gb", [1, 2], F32, isOutput=False)            # [gamma*inv_sqrt_dk, bias]
    idb_ext = dp("identr", [DK, DK], F32R, isOutput=False)
    onr_ext = dp("onesr", [DK, DK], F32R, isOutput=False)
    out_bias = dp("out_bias", [SH, S], F32, isOutput=True)
    out_stats = dp("out_stats", [1, 16], F32, isOutput=True)
    out_ent = dp("out_ent", [DK, NST], F32, isOutput=True)

    decays = [GAMMA ** (w + 1) for w in range(W)]

    with tile.TileContext(nc) as tc:
        with (
            tc.tile_pool(name="const", bufs=1) as cpool,
            tc.tile_pool(name="stream", bufs=3) as spool,
            tc.tile_pool(name="work", bufs=2) as wpool,
            tc.tile_pool(name="obuf", bufs=2) as opool,
            tc.tile_pool(name="ps", bufs=1, space="PSUM") as ps,
        ):
            # ---- constants ----
            v_sb = cpool.tile([128, NK, DK], F32R)
            nc.sync.dma_start(out=v_sb[:], in_=v_ext.ap().rearrange("(k p) d -> p k d", p=128))
            vb_sb = cpool.tile([128, NK, DK], BF)
            nc.scalar.copy(vb_sb[:], v_sb[:])
            ktr = cpool.tile([DK, S], F32R)
            nc.sync.dma_start(out=ktr[:], in_=kt_ext[:])
            kts = cpool.tile([DK, S], F32R)
            nc.scalar.mul(kts[:], ktr[:], INV_SQRT_DK)
            qt_r = cpool.tile([DK, SH], F32R)
            nc.sync.dma_start(out=qt_r[:], in_=qt_ext[:])
            w1tr = []
            for fc in range(2):
                t_r = cpool.tile([DK, D_MLP], F32R, tag=f"w1t{fc}", name=f"w1t{fc}")
                nc.sync.dma_start(out=t_r[:], in_=w1t_ext[fc])
                w1tr.append(t_r)
            b1_sb = []
            w2tr = []
            for mc in range(2):
                t_bias = cpool.tile([DK, 1], F32, tag=f"b1{mc}", name=f"b1{mc}")
                nc.sync.dma_start(out=t_bias[:], in_=b1_ext[mc])
                b1_sb.append(t_bias)
                t_r = cpool.tile([DK, 1], F32, tag=f"w2t{mc}", name=f"w2t{mc}")
                nc.sync.dma_start(out=t_r[:], in_=w2t_ext[mc])
                w2tr.append(t_r)
            wpt_r = cpool.tile([DK, DK], F32R)
            nc.sync.dma_start(out=wpt_r[:], in_=wpt_ext[:])
            wmt_r = cpool.tile([DK, DK], F32R)
            nc.sync.dma_start(out=wmt_r[:], in_=wmt_ext[:])
            gb_sb = cpool.tile([1, 2], F32)
            nc.sync.dma_start(out=gb_sb[:], in_=gb_ext[:])
            ident_r = cpool.tile([DK, DK], F32R)
            nc.sync.dma_start(out=ident_r[:], in_=idb_ext[:])
            ones_r = cpool.tile([DK, DK], F32R)
            nc.sync.dma_start(out=ones_r[:], in_=onr_ext[:])
            ones_gf = cpool.tile([1, 128], F32)
            nc.vector.memset(ones_gf[:], 1.0)
            stats_sb = cpool.tile([1, 16], F32)
            nc.vector.memset(stats_sb[:], 0.0)
            eps_sb = cpool.tile([128, 1], F32)
            nc.vector.memset(eps_sb[:], 1e-9)
            ent_sb = cpool.tile([DK, NST], F32)

            # ---- stage 1: P_w^T = decay_w * [(hist_rev_w + noise'_w) @ V] ----
            pbf = []
            for w in range(W):
                pacc = ps.tile([128, SH], F32, tag="big", bufs=2)
                for k in range(NK):
                    h_t = spool.tile([128, SH], F32R, tag="hist")
                    nc.sync.dma_start(out=h_t[:], in_=histT[w, k * 128:(k + 1) * 128, :])
                    n_t = spool.tile([128, SH], BF, tag="noise")
                    nc.sync.dma_start(out=n_t[:], in_=noiseT[w, k * 128:(k + 1) * 128, :])
                    for c in range(2):
                        nc.tensor.matmul(
                            pacc[:, c * 512:(c + 1) * 512],
                            v_sb[:, k, :],
                            h_t[:, c * 512:(c + 1) * 512],
                            start=(k == 0), stop=False,
                        )
                    for c in range(2):
                        nc.tensor.matmul(
                            pacc[:, c * 512:(c + 1) * 512],
                            vb_sb[:, k, :],
                            n_t[:, c * 512:(c + 1) * 512],
                            start=False, stop=(k == NK - 1),
                        )
                p_r = cpool.tile([128, SH], F32R, tag=f"pf{w}", name=f"pf{w}")
                nc.scalar.mul(p_r[:], pacc[:], decays[w])
                pbf.append(p_r)

            # ---- stage 2: alpha MLP ----
            hb = {}
            for w in range(W):
                pa_ps = ps.tile([128, SH], F32, tag="big", bufs=2)
                for c in range(2):
                    nc.tensor.matmul(
                        pa_ps[:, c * 512:(c + 1) * 512], wpt_r[:],
                        pbf[w][:, c * 512:(c + 1) * 512], start=True, stop=True)
                pab = wpool.tile([128, SH], F32R, tag="projAb")
                nc.scalar.copy(pab[:], pa_ps[:])
                for mc in range(2):
                    h_ps = ps.tile([128, SH], F32, tag="big", bufs=2)
                    for c in range(2):
                        sl = slice(c * 512, (c + 1) * 512)
                        nc.tensor.matmul(h_ps[:, sl], w1tr[0][:, mc * 128:(mc + 1) * 128],
                                         qt_r[:, sl], start=True, stop=False)
                        nc.tensor.matmul(h_ps[:, sl], w1tr[1][:, mc * 128:(mc + 1) * 128],
                                         pab[:, sl], start=False, stop=True)
                    h_r = cpool.tile([128, SH], F32, tag=f"hb{w}{mc}", name=f"hb{w}{mc}")
                    nc.scalar.activation(h_r[:], h_ps[:], AF.Relu, bias=b1_sb[mc][:])
                    hb[(w, mc)] = h_r

            # ---- scores + softmax + entropy per s-tile; transpose alpha ----
            at_sb = cpool.tile([4, SH], F32R)
            atT = [cpool.tile([1, SH], F32R, tag=f"atT{w}", name=f"atT{w}")
                   for w in range(W)]
            for i in range(NST):
                ssl = slice(i * 128, (i + 1) * 128)
                sc_ps = ps.tile([128, 4], F32, tag="smallA", bufs=2)
                for w in range(W):
                    for mc in range(2):
                        nc.tensor.matmul(sc_ps[:, w:w + 1], hb[(w, mc)][:, ssl],
                                         w2tr[mc][:], start=(mc == 0), stop=(mc == 1))
                mx = wpool.tile([128, 1], F32, tag="mx")
                nc.vector.tensor_reduce(mx[:], sc_ps[:, 0:3], mybir.AxisListType.X, ALU.max)
                negmx = wpool.tile([128, 1], F32, tag="negmx")
                nc.vector.tensor_scalar_mul(negmx[:], mx[:], -1.0)
                ex = wpool.tile([128, 3], F32, tag="ex")
                nc.scalar.activation(ex[:], sc_ps[:, 0:3], AF.Exp, bias=negmx[:])
                sm = wpool.tile([128, 1], F32, tag="sm")
                nc.vector.tensor_reduce(sm[:], ex[:], mybir.AxisListType.X, ALU.add)
                rs = wpool.tile([128, 1], F32, tag="rs")
                nc.vector.reciprocal(rs[:], sm[:])
                al_f = wpool.tile([128, 3], F32, tag="al_f")
                nc.vector.tensor_scalar_mul(al_f[:], ex[:], rs[:])
                al_r = wpool.tile([128, 3], F32R, tag="al_r")
                nc.scalar.copy(al_r[:], al_f[:])
                # entropy partial: sum_w alpha*ln(alpha+1e-9)  (host negates)
                lnx = wpool.tile([128, 3], F32, tag="lnx")
                nc.scalar.activation(lnx[:], al_f[:], AF.Ln, bias=eps_sb[:])
                tln = wpool.tile([128, 3], F32, tag="tln")
                nc.vector.tensor_mul(tln[:], al_f[:], lnx[:])
                nc.vector.tensor_reduce(ent_sb[:, i:i + 1], tln[:], mybir.AxisListType.X, ALU.add)
                # alpha^T [3,128] via PE transpose with identity
                at_ps = ps.tile([4, 128], F32, tag="smallA", bufs=2)
                nc.tensor.matmul(at_ps[0:3, :], al_r[:], ident_r[:],
                                 start=True, stop=True)
                nc.scalar.copy(at_sb[0:3, ssl], at_ps[0:3, :])

            for w in range(W):
                nc.sync.dma_start(out=atT[w][:], in_=at_sb[w:w + 1, :])

            # ---- MV = sum_w alpha_w * P_w ; gate; gated ----
            gat = []
            for c in range(2):
                sl = slice(c * 512, (c + 1) * 512)
                abb = []
                for w in range(W):
                    ab_ps = ps.tile([128, 512], F32, tag="smallB", bufs=2)
                    nc.tensor.matmul(ab_ps[:], ones_r[0:1, :], atT[w][0:1, sl], start=True, stop=True)
                    a_r = wpool.tile([128, 512], F32R, tag=f"abb{w}", name=f"abb{w}")
                    nc.scalar.copy(a_r[:], ab_ps[:])
                    abb.append(a_r)
                mvb = wpool.tile([128, 512], F32R, tag="mvb")
                nc.vector.tensor_mul(mvb[:], pbf[0][:, sl], abb[0][:])
                tmp = wpool.tile([128, 512], F32R, tag="mvtmp")
                nc.vector.tensor_mul(tmp[:], pbf[1][:, sl], abb[1][:])
                nc.vector.tensor_add(mvb[:], mvb[:], tmp[:])
                tmp2 = wpool.tile([128, 512], F32R, tag="mvtmp")
                nc.vector.tensor_mul(tmp2[:], pbf[2][:, sl], abb[2][:])
                nc.vector.tensor_add(mvb[:], mvb[:], tmp2[:])
                # M_proj^T, M_transformed^T
                mp_ps = ps.tile([128, 512], F32, tag="smallB", bufs=2)
                nc.tensor.matmul(mp_ps[:], wpt_r[:], mvb[:], start=True, stop=True)
                mt_ps = ps.tile([128, 512], F32, tag="smallB", bufs=2)
                nc.tensor.matmul(mt_ps[:], wmt_r[:], mvb[:], start=True, stop=True)
                mt_r = wpool.tile([128, 512], F32R, tag=f"mt{c}", name=f"mt{c}")
                nc.vector.tensor_copy(mt_r[:], mt_ps[:])
                # sim + G
                prod = wpool.tile([128, 512], F32R, tag="prod")
                nc.vector.tensor_mul(prod[:], qt_r[:, sl], mp_ps[:])
                sim_ps = ps.tile([1, 512], F32, tag="smallB", bufs=2)
                nc.tensor.matmul(sim_ps[:], ones_r[:, 0:1], prod[:], start=True, stop=True)
                G = wpool.tile([1, 512], F32, tag=f"G{c}")
                nc.scalar.activation(G[:], sim_ps[:], AF.Sigmoid,
                                     bias=gb_sb[0:1, 1:2], scale=gb_sb[0:1, 0:1],
                                     accum_out=stats_sb[0:1, c:c + 1])
                g2s = wpool.tile([1, 512], F32, tag="g2s")
                nc.vector.scalar_tensor_tensor(g2s[:], G[:], 1.0, G[:],
                                               ALU.mult, ALU.mult,
                                               accum_out=stats_sb[0:1, 2 + c:3 + c])
                # G broadcast + gated
                gb_ps = ps.tile([128, 512], F32, tag="smallB", bufs=2)
                nc.tensor.matmul(gb_ps[:], ones_gf[:], G[:], start=True, stop=True)
                g_t = cpool.tile([128, 512], F32R, tag=f"gat{c}", name=f"gat{c}")
                nc.vector.tensor_mul(g_t[:], mt_r[:], gb_ps[:])
                gat.append(g_t)
                # norm stats
                sq = wpool.tile([128, 512], F32R, tag="sq")
                nc.vector.tensor_mul(sq[:], g_t[:], g_t[:])
                nq_ps = ps.tile([1, 512], F32, tag="smallB", bufs=2)
                nc.tensor.matmul(nq_ps[:], ones_r[:, 0:1], sq[:], start=True, stop=True)
                nrm = wpool.tile([1, 512], F32, tag="nrm")
                nc.scalar.activation(nrm[:], nq_ps[:], AF.Sqrt,
                                     accum_out=stats_sb[0:1, 4 + c:5 + c])

            # ---- stage 3: memory_bias = gated @ (K^T/sqrt(dk)) ----
            for i in range(NST):
                ob = opool.tile([128, S], F32, tag="ob")
                gsl = gat[i // 4][:, (i % 4) * 128:(i % 4 + 1) * 128]
                for tc_ in range(4):
                    tsl = slice(tc_ * 512, (tc_ + 1) * 512)
                    bo_ps = ps.tile([128, 512], F32, tag="big", bufs=2)
                    nc.tensor.matmul(bo_ps[:], gsl, kts[:, tsl], start=True, stop=True)
                    nc.vector.tensor_copy(ob[:, tsl], bo_ps[:])
                nc.sync.dma_start(out=out_bias[i * 128:(i + 1) * 128, :], in_=ob[:])

            nc.sync.dma_start(out=out_stats[:], in_=stats_sb[:])
            nc.sync.dma_start(out=out_ent[:], in_=ent_sb[:])
    nc.compile()
    return nc


_NC_CACHE = None
_NOISE_CACHE = None


def _get_nc():
    global _NC_CACHE
    if _NC_CACHE is None:
        _NC_CACHE = _build()
    return _NC_CACHE


def _get_noise_shards():
    """Per-core noiseT arrays: [w, t, s_shard] bf16, pre-scaled by eps/decay_w."""
    global _NOISE_CACHE
    if _NOISE_CACHE is None:
        import jax
        import jax.numpy as jnp
        with jax.default_device(jax.devices("cpu")[0]):
            noise = np.asarray(jax.random.uniform(
                jax.random.key(42), (W, B, S, S), jnp.float32))
        shards = []
        for core in range(NC):
            b, h = core // 2, core % 2
            sl = slice(h * SH, (h + 1) * SH)
            nt = np.empty((W, S, SH), dtype=BF16)
            for w in range(W):
                scale = EPSILON / (GAMMA ** (w + 1))
                nt[w] = (noise[w, b, sl, :].T * scale).astype(BF16)
            shards.append(nt)
        del noise
        _NOISE_CACHE = shards
    return _NOISE_CACHE


def kernel(Q, K, V, attention_history, W1, b1, W2, b2, w_mem, w_proj,
           gamma_g, bias_g, relative_layer_idx):
    Q = np.asarray(Q, np.float32)
    K = np.asarray(K, np.float32)
    V = np.asarray(V, np.float32)
    hist = np.asarray(attention_history, np.float32)
    if int(relative_layer_idx) <= 1 or hist.shape[0] == 0:
        z = np.zeros((B,), np.float32)
        return (np.zeros((B, S, S), np.float32), z, z, z, z)

    W1 = np.asarray(W1, np.float32)
    b1 = np.asarray(b1, np.float32)
    W2 = np.asarray(W2, np.float32)
    w_mem = np.asarray(w_mem, np.float32)
    w_proj = np.asarray(w_proj, np.float32)
    gamma_gf = float(np.asarray(gamma_g))
    bias_gf = float(np.asarray(bias_g))

    noise_shards = _get_noise_shards()
    nc = _get_nc()

    w1t = np.ascontiguousarray(W1.T.reshape(2, DK, D_MLP))
    b1r = np.ascontiguousarray(b1.reshape(2, DK, 1))
    w2t = np.ascontiguousarray(W2.T.reshape(2, DK, 1))
    wpt = np.ascontiguousarray(w_proj.T)
    wmt = np.ascontiguousarray(w_mem.T)
    gb = np.array([[gamma_gf * INV_SQRT_DK, bias_gf]], np.float32)
    identr = np.eye(DK, dtype=np.float32)
    onesr = np.ones((DK, DK), np.float32)

    in_maps = []
    for core in range(NC):
        b, h = core // 2, core % 2
        sl = slice(h * SH, (h + 1) * SH)
        histT = np.empty((W, S, SH), np.float32)
        for w in range(W):
            histT[w] = hist[W - 1 - w, b, sl, :].T
        in_maps.append({
            "histT": histT,
            "noiseT": noise_shards[core],
            "v": np.ascontiguousarray(V[b]),
            "kt": np.ascontiguousarray(K[b].T),
            "qt": np.ascontiguousarray(Q[b, sl, :].T),
            "w1t": w1t, "b1r": b1r, "w2t": w2t,
            "wpt": wpt, "wmt": wmt, "gb": gb, "identr": identr, "onesr": onesr,
        })

    res = run_bass_kernel_spmd(nc, in_maps, list(range(NC)), trace=False)

    memory_bias = np.empty((B, S, S), np.float32)
    gi = np.empty(B, np.float32)
    gv = np.empty(B, np.float32)
    ad = np.empty(B, np.float32)
    mc = np.empty(B, np.float32)
    for b in range(B):
        sg = sg2 = snorm = sent = 0.0
        for h in range(2):
            r = res.results[2 * b + h]
            memory_bias[b, h * SH:(h + 1) * SH, :] = r["out_bias"]
            st = r["out_stats"][0]
            sg += float(st[0] + st[1])
            sg2 += float(st[2] + st[3])
            snorm += float(st[4] + st[5])
            sent += float(r["out_ent"].sum())
        gi[b] = sg / S
        gv[b] = (sg2 - sg * sg / S) / (S - 1)
        ad[b] = -sent / S
        mc[b] = snorm / S
    using_memory = np.ones(B, np.float32)
    return (memory_bias, gi, gv, ad, mc, using_memory)


# revision 19
# speedup vs baseline: 1.1212x; 1.1212x over previous
# Distributed Trainium2 Bass kernel for nn_AMRPACore (sparse_attention).
#
# Sharding: 8 cores = (batch b in 0..3) x (query-row half h in 0..1).
# Each core handles rows [h*1024, (h+1)*1024) of batch b and produces its
# memory_bias shard plus partial reductions for the scalar outputs.
#
# Math notes (vs the reference):
#  - proj_vals P_w = decayed_w @ V with decayed_w = g^w+1 * hist[W-1-w] + noise_w.
#    Computed as g^(w+1) * [(hist_rev_w + noise_w/g^(w+1)) @ V]; the noise term
#    is an input-independent constant (jax threefry key 42), generated host-side
#    once, pre-transposed and pre-scaled, stored bf16.
#  - M (B,S,S) is never materialized: MV = einsum('bsw,wbst,btd->bsd') reduces to
#    MV = sum_w alpha_w * P_w.
#  - b2 only shifts all alpha logits equally -> cancels in softmax; dropped.
#  - sim_score scaling (1/sqrt(dk)) is folded into the sigmoid scale; the
#    memory_bias 1/sqrt(dk) is folded into K^T.
import sys
for _p in ('/opt/trn_rl_repo', '/root/.axon_site/_ro/trn_rl_repo'):
    if _p not in sys.path:
        sys.path.insert(0, _p)

import math
import numpy as np
import ml_dtypes

from concourse import bacc, bass, tile, mybir
from concourse.bass_utils import run_bass_kernel_spmd

BF16 = ml_dtypes.bfloat16
F32 = mybir.dt.float32
F32R = mybir.dt.float32r
BF = mybir.dt.bfloat16

B, S, DK = 4, 2048, 128
W = 3
D_MLP = 256
GAMMA = 0.9
EPSILON = 0.01
INV_SQRT_DK = 1.0 / math.sqrt(DK)
SH = S // 2          # rows per core
NK = S // 128        # 16 contraction tiles
NST = SH // 128      # 8 s-tiles per core
NC = 8               # cores
AF = mybir.ActivationFunctionType
ALU = mybir.AluOpType


def _build():
    nc = bacc.Bacc("TRN2", target_bir_lowering=False, debug=False, num_devices=NC)
    dp = nc.declare_dram_parameter
    histT = dp("histT", [W, S, SH], F32, isOutput=False)      # [w, t, s] f32
    noiseT = dp("noiseT", [W, S, SH], BF, isOutput=False)     # [w, t, s] bf16 (pre-scaled)
    v_ext = dp("v", [S, DK], F32, isOutput=False)
    kt_ext = dp("kt", [DK, S], F32R, isOutput=False)           # K^T
    qt_ext = dp("qt", [DK, SH], F32R, isOutput=False)          # Q shard ^T
    w1t_ext = dp("w1t", [2, DK, D_MLP], F32R, isOutput=False)  # W1^T f-chunks
    b1_ext = dp("b1r", [2, DK, 1], F32, isOutput=False)
    w2t_ext = dp("w2t", [2, DK, 1], BF, isOutput=False)      # W2^T m-chunks
    wpt_ext = dp("wpt", [DK, DK], F32R, isOutput=False)        # w_proj^T
    wmt_ext = dp("wmt", [DK, DK], F32R, isOutput=False)        # w_mem^T
    gb_ext = dp("gb", [1, 2], F32, isOutput=False)            # [gamma*inv_sqrt_dk, bias]
    idb_ext = dp("identr", [DK, DK], F32R, isOutput=False)
    onr_ext = dp("onesr", [DK, DK], F32R, isOutput=False)
    out_bias = dp("out_bias", [SH, S], F32, isOutput=True)
    out_stats = dp("out_stats", [1, 16], F32, isOutput=True)
    out_ent = dp("out_ent", [DK, NST], F32, isOutput=True)

    decays = [GAMMA ** (w + 1) for w in range(W)]

    with tile.TileContext(nc) as tc:
        with (
            tc.tile_pool(name="const", bufs=1) as cpool,
            tc.tile_pool(name="stream", bufs=3) as spool,
            tc.tile_pool(name="work", bufs=2) as wpool,
            tc.tile_pool(name="obuf", bufs=2) as opool,
            tc.tile_pool(name="ps", bufs=1, space="PSUM") as ps,
        ):
            # ---- constants ----
            v_sb = cpool.tile([128, NK, DK], F32)
            nc.sync.dma_start(out=v_sb[:], in_=v_ext.ap().rearrange("(k p) d -> p k d", p=128))
            vb_sb = cpool.tile([128, NK, DK], BF)
            nc.scalar.copy(vb_sb[:], v_sb[:])
            ktr = cpool.tile([DK, S], F32R)
            nc.sync.dma_start(out=ktr[:], in_=kt_ext[:])
            kts = cpool.tile([DK, S], F32R)
            nc.scalar.mul(kts[:], ktr[:], INV_SQRT_DK)
            qt_r = cpool.tile([DK, SH], F32R)
            nc.sync.dma_start(out=qt_r[:], in_=qt_ext[:])
            w1tr = []
            for fc in range(2):
                t_r = cpool.tile([DK, D_MLP], F32R, tag=f"w1t{fc}", name=f"w1t{fc}")
                nc.sync.dma_start(out=t_r[:], in_=w1t_ext[fc])
                w1tr.append(t_r)
            b1_sb = []
            w2tr = []
            for mc in range(2):
                t_bias = cpool.tile([DK, 1], F32, tag=f"b1{mc}", name=f"b1{mc}")
                nc.sync.dma_start(out=t_bias[:], in_=b1_ext[mc])
                b1_sb.append(t_bias)
                t_r = cpool.tile([DK, 1], BF, tag=f"w2t{mc}", name=f"w2t{mc}")
                nc.sync.dma_start(out=t_r[:], in_=w2t_ext[mc])
                w2tr.append(t_r)
            wpt_r = cpool.tile([DK, DK], F32R)
            nc.sync.dma_start(out=wpt_r[:], in_=wpt_ext[:])
            wmt_r = cpool.tile([DK, DK], F32R)
            nc.sync.dma_start(out=wmt_r[:], in_=wmt_ext[:])
            gb_sb = cpool.tile([1, 2], F32)
            nc.sync.dma_start(out=gb_sb[:], in_=gb_ext[:])
            ident_r = cpool.tile([DK, DK], F32R)
            nc.sync.dma_start(out=ident_r[:], in_=idb_ext[:])
            ones_r = cpool.tile([DK, DK], F32R)
            nc.sync.dma_start(out=ones_r[:], in_=onr_ext[:])
            ones_b = cpool.tile([1, 128], BF)
            nc.vector.memset(ones_b[:], 1.0)
            ones_gf = cpool.tile([1, 128], F32)
            nc.vector.memset(ones_gf[:], 1.0)
            stats_sb = cpool.tile([1, 16], F32)
            nc.vector.memset(stats_sb[:], 0.0)
            eps_sb = cpool.tile([128, 1], F32)
            nc.vector.memset(eps_sb[:], 1e-9)
            ent_sb = cpool.tile([DK, NST], F32)

            # ---- stage 1: P_w^T = decay_w * [(hist_rev_w + noise'_w) @ V] ----
            pbf = []
            for w in range(W):
                pacc = ps.tile([128, SH], F32, tag="big", bufs=2)
                for k in range(NK):
                    n_t = spool.tile([128, SH], BF, tag="noise")
                    nc.sync.dma_start(out=n_t[:], in_=noiseT[w, k * 128:(k + 1) * 128, :])
                    h_t = spool.tile([128, SH], BF, tag="hist")
                    nc.gpsimd.dma_start(out=h_t[:], in_=histT[w, k * 128:(k + 1) * 128, :])
                    m_t = spool.tile([128, SH], BF, tag="merged")
                    nc.vector.tensor_add(m_t[:], h_t[:], n_t[:])
                    for c in range(2):
                        nc.tensor.matmul(
                            pacc[:, c * 512:(c + 1) * 512],
                            vb_sb[:, k, :],
                            m_t[:, c * 512:(c + 1) * 512],
                            start=(k == 0), stop=(k == NK - 1),
                        )
                p_r = cpool.tile([128, SH], F32R, tag=f"pf{w}", name=f"pf{w}")
                nc.scalar.mul(p_r[:], pacc[:], decays[w])
                pbf.append(p_r)

            # ---- stage 2: alpha MLP ----
            hb = {}
            for w in range(W):
                pa_ps = ps.tile([128, SH], F32, tag="big", bufs=2)
                for c in range(2):
                    nc.tensor.matmul(
                        pa_ps[:, c * 512:(c + 1) * 512], wpt_r[:],
                        pbf[w][:, c * 512:(c + 1) * 512], start=True, stop=True)
                pab = wpool.tile([128, SH], F32R, tag="projAb")
                nc.scalar.copy(pab[:], pa_ps[:])
                for mc in range(2):
                    h_ps = ps.tile([128, SH], F32, tag="big", bufs=2)
                    for c in range(2):
                        sl = slice(c * 512, (c + 1) * 512)
                        nc.tensor.matmul(h_ps[:, sl], w1tr[0][:, mc * 128:(mc + 1) * 128],
                                         qt_r[:, sl], start=True, stop=False)
                        nc.tensor.matmul(h_ps[:, sl], w1tr[1][:, mc * 128:(mc + 1) * 128],
                                         pab[:, sl], start=False, stop=True)
                    h_r = cpool.tile([128, SH], BF, tag=f"hb{w}{mc}", name=f"hb{w}{mc}")
                    nc.scalar.activation(h_r[:], h_ps[:], AF.Relu, bias=b1_sb[mc][:])
                    hb[(w, mc)] = h_r

            # ---- scores + softmax + entropy per s-tile; transpose alpha ----
            at_sb = cpool.tile([4, SH], F32R)
            atT = [cpool.tile([1, SH], F32R, tag=f"atT{w}", name=f"atT{w}")
                   for w in range(W)]
            for i in range(NST):
                ssl = slice(i * 128, (i + 1) * 128)
                sc_ps = ps.tile([128, 4], F32, tag="smallA", bufs=2)
                for w in range(W):
                    for mc in range(2):
                        nc.tensor.matmul(sc_ps[:, w:w + 1], hb[(w, mc)][:, ssl],
                                         w2tr[mc][:], start=(mc == 0), stop=(mc == 1))
                mx = wpool.tile([128, 1], F32, tag="mx")
                nc.vector.tensor_reduce(mx[:], sc_ps[:, 0:3], mybir.AxisListType.X, ALU.max)
                negmx = wpool.tile([128, 1], F32, tag="negmx")
                nc.vector.tensor_scalar_mul(negmx[:], mx[:], -1.0)
                ex = wpool.tile([128, 3], F32, tag="ex")
                nc.scalar.activation(ex[:], sc_ps[:, 0:3], AF.Exp, bias=negmx[:])
                sm = wpool.tile([128, 1], F32, tag="sm")
                nc.vector.tensor_reduce(sm[:], ex[:], mybir.AxisListType.X, ALU.add)
                rs = wpool.tile([128, 1], F32, tag="rs")
                nc.vector.reciprocal(rs[:], sm[:])
                al_f = wpool.tile([128, 3], F32, tag="al_f")
                nc.vector.tensor_scalar_mul(al_f[:], ex[:], rs[:])
                al_r = wpool.tile([128, 3], F32R, tag="al_r")
                nc.scalar.copy(al_r[:], al_f[:])
                # entropy partial: sum_w alpha*ln(alpha+1e-9)  (host negates)
                lnx = wpool.tile([128, 3], F32, tag="lnx")
                nc.scalar.activation(lnx[:], al_f[:], AF.Ln, bias=eps_sb[:])
                tln = wpool.tile([128, 3], F32, tag="tln")
                nc.vector.tensor_mul(tln[:], al_f[:], lnx[:])
                nc.vector.tensor_reduce(ent_sb[:, i:i + 1], tln[:], mybir.AxisListType.X, ALU.add)
                # alpha^T [3,128] via PE transpose with identity
                at_ps = ps.tile([4, 128], F32, tag="smallA", bufs=2)
                nc.tensor.matmul(at_ps[0:3, :], al_r[:], ident_r[:],
                                 start=True, stop=True)
                nc.scalar.copy(at_sb[0:3, ssl], at_ps[0:3, :])

            for w in range(W):
                nc.sync.dma_start(out=atT[w][:], in_=at_sb[w:w + 1, :])

            # ---- MV = sum_w alpha_w * P_w ; gate; gated ----
            gat = []
            for c in range(2):
                sl = slice(c * 512, (c + 1) * 512)
                abb = []
                for w in range(W):
                    ab_ps = ps.tile([128, 512], F32, tag="smallB", bufs=2)
                    nc.tensor.matmul(ab_ps[:], ones_r[0:1, :], atT[w][0:1, sl], start=True, stop=True)
                    a_r = wpool.tile([128, 512], F32R, tag=f"abb{w}", name=f"abb{w}")
                    nc.scalar.copy(a_r[:], ab_ps[:])
                    abb.append(a_r)
                mvb = wpool.tile([128, 512], F32R, tag="mvb")
                nc.vector.tensor_mul(mvb[:], pbf[0][:, sl], abb[0][:])
                tmp = wpool.tile([128, 512], F32R, tag="mvtmp")
                nc.vector.tensor_mul(tmp[:], pbf[1][:, sl], abb[1][:])
                nc.vector.tensor_add(mvb[:], mvb[:], tmp[:])
                tmp2 = wpool.tile([128, 512], F32R, tag="mvtmp")
                nc.vector.tensor_mul(tmp2[:], pbf[2][:, sl], abb[2][:])
                nc.vector.tensor_add(mvb[:], mvb[:], tmp2[:])
                # M_proj^T, M_transformed^T
                mp_ps = ps.tile([128, 512], F32, tag="smallB", bufs=2)
                nc.tensor.matmul(mp_ps[:], wpt_r[:], mvb[:], start=True, stop=True)
                mt_ps = ps.tile([128, 512], F32, tag="smallB", bufs=2)
                nc.tensor.matmul(mt_ps[:], wmt_r[:], mvb[:], start=True, stop=True)
                mt_r = wpool.tile([128, 512], F32R, tag=f"mt{c}", name=f"mt{c}")
                nc.vector.tensor_copy(mt_r[:], mt_ps[:])
                # sim + G
                prod = wpool.tile([128, 512], F32R, tag="prod")
                nc.vector.tensor_mul(prod[:], qt_r[:, sl], mp_ps[:])
                sim_ps = ps.tile([1, 512], F32, tag="smallB", bufs=2)
                nc.tensor.matmul(sim_ps[:], ones_r[:, 0:1], prod[:], start=True, stop=True)
                G = wpool.tile([1, 512], F32, tag=f"G{c}")
                nc.scalar.activation(G[:], sim_ps[:], AF.Sigmoid,
                                     bias=gb_sb[0:1, 1:2], scale=gb_sb[0:1, 0:1],
                                     accum_out=stats_sb[0:1, c:c + 1])
                g2s = wpool.tile([1, 512], F32, tag="g2s")
                nc.vector.scalar_tensor_tensor(g2s[:], G[:], 1.0, G[:],
                                               ALU.mult, ALU.mult,
                                               accum_out=stats_sb[0:1, 2 + c:3 + c])
                # G broadcast + gated
                gb_ps = ps.tile([128, 512], F32, tag="smallB", bufs=2)
                nc.tensor.matmul(gb_ps[:], ones_gf[:], G[:], start=True, stop=True)
                g_t = cpool.tile([128, 512], F32R, tag=f"gat{c}", name=f"gat{c}")
                nc.vector.tensor_mul(g_t[:], mt_r[:], gb_ps[:])
                gat.append(g_t)
                # norm stats
                sq = wpool.tile([128, 512], F32R, tag="sq")
                nc.vector.tensor_mul(sq[:], g_t[:], g_t[:])
                nq_ps = ps.tile([1, 512], F32, tag="smallB", bufs=2)
                nc.tensor.matmul(nq_ps[:], ones_r[:, 0:1], sq[:], start=True, stop=True)
                nrm = wpool.tile([1, 512], F32, tag="nrm")
                nc.scalar.activation(nrm[:], nq_ps[:], AF.Sqrt,
                                     accum_out=stats_sb[0:1, 4 + c:5 + c])

            # ---- stage 3: memory_bias = gated @ (K^T/sqrt(dk)) ----
            for i in range(NST):
                ob = opool.tile([128, S], F32, tag="ob")
                gsl = gat[i // 4][:, (i % 4) * 128:(i % 4 + 1) * 128]
                for tc_ in range(4):
                    tsl = slice(tc_ * 512, (tc_ + 1) * 512)
                    bo_ps = ps.tile([128, 512], F32, tag="big", bufs=2)
                    nc.tensor.matmul(bo_ps[:], gsl, kts[:, tsl], start=True, stop=True)
                    nc.vector.tensor_copy(ob[:, tsl], bo_ps[:])
                nc.sync.dma_start(out=out_bias[i * 128:(i + 1) * 128, :], in_=ob[:])

            nc.sync.dma_start(out=out_stats[:], in_=stats_sb[:])
            nc.sync.dma_start(out=out_ent[:], in_=ent_sb[:])
    nc.compile()
    return nc


_NC_CACHE = None
_NOISE_CACHE = None


def _get_nc():
    global _NC_CACHE
    if _NC_CACHE is None:
        _NC_CACHE = _build()
    return _NC_CACHE


def _get_noise_shards():
    """Per-core noiseT arrays: [w, t, s_shard] bf16, pre-scaled by eps/decay_w."""
    global _NOISE_CACHE
    if _NOISE_CACHE is None:
        import jax
        import jax.numpy as jnp
        with jax.default_device(jax.devices("cpu")[0]):
            noise = np.asarray(jax.random.uniform(
                jax.random.key(42), (W, B, S, S), jnp.float32))
        shards = []
        for core in range(NC):
            b, h = core // 2, core % 2
            sl = slice(h * SH, (h + 1) * SH)
            nt = np.empty((W, S, SH), dtype=BF16)
            for w in range(W):
                scale = EPSILON / (GAMMA ** (w + 1))
                nt[w] = (noise[w, b, sl, :].T * scale).astype(BF16)
            shards.append(nt)
        del noise
        _NOISE_CACHE = shards
    return _NOISE_CACHE


def kernel(Q, K, V, attention_history, W1, b1, W2, b2, w_mem, w_proj,
           gamma_g, bias_g, relative_layer_idx):
    Q = np.asarray(Q, np.float32)
    K = np.asarray(K, np.float32)
    V = np.asarray(V, np.float32)
    hist = np.asarray(attention_history, np.float32)
    if int(relative_layer_idx) <= 1 or hist.shape[0] == 0:
        z = np.zeros((B,), np.float32)
        return (np.zeros((B, S, S), np.float32), z, z, z, z)

    W1 = np.asarray(W1, np.float32)
    b1 = np.asarray(b1, np.float32)
    W2 = np.asarray(W2, np.float32)
    w_mem = np.asarray(w_mem, np.float32)
    w_proj = np.asarray(w_proj, np.float32)
    gamma_gf = float(np.asarray(gamma_g))
    bias_gf = float(np.asarray(bias_g))

    noise_shards = _get_noise_shards()
    nc = _get_nc()

    w1t = np.ascontiguousarray(W1.T.reshape(2, DK, D_MLP))
    b1r = np.ascontiguousarray(b1.reshape(2, DK, 1))
    w2t = np.ascontiguousarray(W2.T.reshape(2, DK, 1)).astype(BF16)
    wpt = np.ascontiguousarray(w_proj.T)
    wmt = np.ascontiguousarray(w_mem.T)
    gb = np.array([[gamma_gf * INV_SQRT_DK, bias_gf]], np.float32)
    identr = np.eye(DK, dtype=np.float32)
    onesr = np.ones((DK, DK), np.float32)

    in_maps = []
    for core in range(NC):
        b, h = core // 2, core % 2
        sl = slice(h * SH, (h + 1) * SH)
        histT = np.empty((W, S, SH), np.float32)
        for w in range(W):
            histT[w] = hist[W - 1 - w, b, sl, :].T
        in_maps.append({
            "histT": histT,
            "noiseT": noise_shards[core],
            "v": np.ascontiguousarray(V[b]),
            "kt": np.ascontiguousarray(K[b].T),
            "qt": np.ascontiguousarray(Q[b, sl, :].T),
            "w1t": w1t, "b1r": b1r, "w2t": w2t,
            "wpt": wpt, "wmt": wmt, "gb": gb, "identr": identr, "onesr": onesr,
        })

    res = run_bass_kernel_spmd(nc, in_maps, list(range(NC)), trace=False)

    memory_bias = np.empty((B, S, S), np.float32)
    gi = np.empty(B, np.float32)
    gv = np.empty(B, np.float32)
    ad = np.empty(B, np.float32)
    mc = np.empty(B, np.float32)
    for b in range(B):
        sg = sg2 = snorm = sent = 0.0
        for h in range(2):
            r = res.results[2 * b + h]
            memory_bias[b, h * SH:(h + 1) * SH, :] = r["out_bias"]
            st = r["out_stats"][0]
            sg += float(st[0] + st[1])
            sg2 += float(st[2] + st[3])
            snorm += float(st[4] + st[5])
            sent += float(r["out_ent"].sum())
        gi[b] = sg / S
        gv[b] = (sg2 - sg * sg / S) / (S - 1)
        ad[b] = -sent / S
        mc[b] = snorm / S
    using_memory = np.ones(B, np.float32)
    return (memory_bias, gi, gv, ad, mc, using_memory)
